# revision 7
# baseline (speedup 1.0000x reference)
"""Trainium2 Bass kernel for nn_CDAATRACK histogram-binning priors.

Computation per batch image:
  - fore/back rectangle masks on the 128x128 template from rounded xywh bbox
  - 4096-bin joint color histogram + 256-bin depth histogram of the template,
    masked by fore/back -> converted to per-bin prior tables
  - per-pixel table lookup on the 384x384 search image -> [4, 384, 384] priors

Device mapping (8 NeuronCores, 8 images per core):
  - histograms: one-hot matmuls on the tensor engine (pixels on the
    contraction dim, bins = 128lo x 32hi for color, 16lo x 16hi for depth)
  - prior tables: elementwise DVE math on the [lo, hi] histogram grid,
    dumped to DRAM in lo-major bin order (so the dump is contiguous),
    read back replicated to partitions {0,16,...,112} for ap_gather
  - per-pixel lookups: GPSIMD ap_gather (d=2 -> fore+back in one index)
  - per-pixel bin indices: DVE integer ops (and/shift/add)
"""

import numpy as np

import concourse.bass as bass
import concourse.bacc as bacc
import concourse.mybir as mybir
import concourse.tile as tile
from concourse import bass_utils

F32 = mybir.dt.float32
BF16 = mybir.dt.bfloat16
I32 = mybir.dt.int32
I16 = mybir.dt.int16
AL = mybir.AluOpType

B, Ht, Wt, Hs, Ws = 64, 128, 128, 384, 384
NCORES = 8
BPC = B // NCORES          # images per core
NPX = Hs * Ws              # search pixels per image
PPP = NPX // 128           # search pixels per partition (1152)
NT = Ht * Wt               # template pixels
S_COLS = 144               # search pixels per partition per chunk
NCHUNK = PPP // S_COLS     # chunks per image (8)
NIDX = 16 * S_COLS         # gather indices per core per call
EPS = 1e-5


def _build_nc():
    nc = bacc.Bacc("TRN2", target_bir_lowering=False, debug=False,
                   enable_asserts=False, num_devices=NCORES)

    anno = nc.dram_tensor("anno", [BPC, 4], F32, kind="ExternalInput").ap()
    tcol = nc.dram_tensor("tcol", [BPC, 128, 3 * Wt], I32, kind="ExternalInput").ap()
    tdep = nc.dram_tensor("tdep", [BPC, 128, Wt], I32, kind="ExternalInput").ap()
    scol = nc.dram_tensor("scol", [BPC, 3 * NPX], I32, kind="ExternalInput").ap()
    sdep = nc.dram_tensor("sdep", [BPC, NPX], I32, kind="ExternalInput").ap()
    iota128 = nc.dram_tensor("iota128", [128, 128], BF16, kind="ExternalInput").ap()
    iota32 = nc.dram_tensor("iota32", [128, 32], BF16, kind="ExternalInput").ap()
    iota16 = nc.dram_tensor("iota16", [128, 16], BF16, kind="ExternalInput").ap()
    iotap = nc.dram_tensor("iotap", [128, 1], F32, kind="ExternalInput").ap()
    iotac = nc.dram_tensor("iotac", [128, 128], F32, kind="ExternalInput").ap()
    ones1 = nc.dram_tensor("ones1", [1, 128], F32, kind="ExternalInput").ap()
    ones128 = nc.dram_tensor("ones128", [128, 1], F32, kind="ExternalInput").ap()
    tblc_d = nc.dram_tensor("tblc_d", [BPC, 8192], F32, kind="Internal").ap()
    tbld_d = nc.dram_tensor("tbld_d", [BPC, 512], F32, kind="Internal").ap()
    out = nc.dram_tensor("out", [BPC, 4, NPX], F32, kind="ExternalOutput").ap()

    v = nc.vector
    with tile.TileContext(nc) as tc:
        with tc.tile_pool(name="consts", bufs=1) as cpool, \
             tc.tile_pool(name="timg", bufs=2) as timg, \
             tc.tile_pool(name="tbig", bufs=1) as tbig, \
             tc.tile_pool(name="gtbl", bufs=2) as gtbl, \
             tc.tile_pool(name="spx", bufs=3) as spx, \
             tc.tile_pool(name="gout", bufs=2) as gpoolo, \
             tc.tile_pool(name="pss", bufs=2, space="PSUM") as pss, \
             tc.tile_pool(name="psb", bufs=2, space="PSUM") as psb:

            io128 = cpool.tile([128, 128], BF16)
            nc.sync.dma_start(io128[:], iota128)
            io32 = cpool.tile([128, 32], BF16)
            nc.sync.dma_start(io32[:], iota32)
            io16 = cpool.tile([128, 16], BF16)
            nc.sync.dma_start(io16[:], iota16)
            iop = cpool.tile([128, 1], F32)
            nc.sync.dma_start(iop[:], iotap)
            ioc = cpool.tile([128, 128], F32)
            nc.sync.dma_start(ioc[:], iotac)
            on1 = cpool.tile([1, 128], F32)
            nc.sync.dma_start(on1[:], ones1)
            on128 = cpool.tile([128, 1], F32)
            nc.sync.dma_start(on128[:], ones128)

            for b in range(BPC):
                # ---------------- template phase ----------------
                tc_t = timg.tile([128, 3 * Wt], I32)
                nc.sync.dma_start(tc_t[:], tcol[b])
                td_t = timg.tile([128, Wt], I32)
                nc.sync.dma_start(td_t[:], tdep[b])

                an_t = timg.tile([1, 4], F32)
                nc.sync.dma_start(an_t[:], anno[b].unsqueeze(0))
                an_i = timg.tile([1, 4], I32)
                v.tensor_copy(an_i[:], an_t[:])            # round f32->i32
                an_f = timg.tile([1, 4], F32)
                v.tensor_copy(an_f[:], an_i[:])
                bb_st = timg.tile([1, 4], F32)             # xmin ymin xmax ymax
                v.tensor_copy(bb_st[:, 0:2], an_f[:, 0:2])
                v.tensor_tensor(bb_st[:, 2:4], an_f[:, 0:2], an_f[:, 2:4], AL.add)
                bb_ps = psb.tile([128, 4], F32, tag="ps_small")
                nc.tensor.matmul(bb_ps[:], on1[:], bb_st[:], start=True, stop=True)

                # fore mask [row=partition, col]
                m1 = timg.tile([128, 1], F32)
                v.tensor_scalar(m1[:], iop[:], bb_ps[:, 1:2], None, op0=AL.is_ge)
                m2 = timg.tile([128, 1], F32)
                v.tensor_scalar(m2[:], iop[:], bb_ps[:, 3:4], None, op0=AL.is_lt)
                mrow = timg.tile([128, 1], F32)
                v.tensor_tensor(mrow[:], m1[:], m2[:], AL.mult)
                c1 = timg.tile([128, 128], F32)
                v.tensor_scalar(c1[:], ioc[:], bb_ps[:, 0:1], None, op0=AL.is_ge)
                c2 = timg.tile([128, 128], F32)
                v.tensor_scalar(c2[:], ioc[:], bb_ps[:, 2:3], None, op0=AL.is_lt)
                fore = timg.tile([128, 128], F32)
                v.tensor_tensor(fore[:], c1[:], c2[:], AL.mult)
                v.tensor_scalar(fore[:], fore[:], mrow[:], None, op0=AL.mult)

                # color lo/hi (lin = r4:g4:b4; lo = low7, hi = high5)
                rch = tc_t[:].rearrange("p (w c) -> p w c", c=3)
                lo_i = timg.tile([128, 128], I32)
                t_a = timg.tile([128, 128], I32)
                v.tensor_scalar(t_a[:], rch[:, :, 1], 112, None, op0=AL.bitwise_and)
                v.tensor_scalar(lo_i[:], rch[:, :, 2], 4, None,
                                op0=AL.logical_shift_right)
                v.tensor_tensor(lo_i[:], lo_i[:], t_a[:], AL.add)
                hi_i = timg.tile([128, 128], I32)
                v.tensor_scalar(t_a[:], rch[:, :, 0], 240, 3,
                                op0=AL.bitwise_and, op1=AL.logical_shift_right)
                v.tensor_scalar(hi_i[:], rch[:, :, 1], 7, None,
                                op0=AL.logical_shift_right)
                v.tensor_tensor(hi_i[:], hi_i[:], t_a[:], AL.add)
                lo_f = timg.tile([128, 128], F32)
                v.tensor_copy(lo_f[:], lo_i[:])
                hi_f = timg.tile([128, 128], F32)
                v.tensor_copy(hi_f[:], hi_i[:])

                # depth lo/hi (lo = low4, hi = high4)
                lod_i = timg.tile([128, 128], I32)
                v.tensor_scalar(lod_i[:], td_t[:], 15, None, op0=AL.bitwise_and)
                hid_i = timg.tile([128, 128], I32)
                v.tensor_scalar(hid_i[:], td_t[:], 4, None,
                                op0=AL.logical_shift_right)
                lod_f = timg.tile([128, 128], F32)
                v.tensor_copy(lod_f[:], lod_i[:])
                hid_f = timg.tile([128, 128], F32)
                v.tensor_copy(hid_f[:], hid_i[:])

                # histograms: accumulate over 128 pixel-columns in 4 quarters
                ps_c = pss.tile([128, 64], F32)
                ps_d = pss.tile([16, 32], F32)
                QC = 16
                for q in range(128 // QC):
                    cs = slice(q * QC, (q + 1) * QC)
                    ohlo = tbig.tile([128, QC, 128], BF16)
                    v.tensor_tensor(
                        ohlo[:],
                        lo_f[:, cs].unsqueeze(2).to_broadcast([128, QC, 128]),
                        io128[:].unsqueeze(1).to_broadcast([128, QC, 128]),
                        AL.is_equal)
                    rhs = tbig.tile([128, QC, 64], BF16)
                    v.tensor_tensor(
                        rhs[:, :, 0:32],
                        hi_f[:, cs].unsqueeze(2).to_broadcast([128, QC, 32]),
                        io32[:].unsqueeze(1).to_broadcast([128, QC, 32]),
                        AL.is_equal)
                    v.tensor_tensor(
                        rhs[:, :, 32:64], rhs[:, :, 0:32],
                        fore[:, cs].unsqueeze(2).to_broadcast([128, QC, 32]),
                        AL.mult)
                    ohlod = tbig.tile([128, QC, 16], BF16)
                    v.tensor_tensor(
                        ohlod[:],
                        lod_f[:, cs].unsqueeze(2).to_broadcast([128, QC, 16]),
                        io16[:].unsqueeze(1).to_broadcast([128, QC, 16]),
                        AL.is_equal)
                    rhsd = tbig.tile([128, QC, 32], BF16)
                    v.tensor_tensor(
                        rhsd[:, :, 0:16],
                        hid_f[:, cs].unsqueeze(2).to_broadcast([128, QC, 16]),
                        io16[:].unsqueeze(1).to_broadcast([128, QC, 16]),
                        AL.is_equal)
                    v.tensor_tensor(
                        rhsd[:, :, 16:32], rhsd[:, :, 0:16],
                        fore[:, cs].unsqueeze(2).to_broadcast([128, QC, 16]),
                        AL.mult)
                    for c in range(QC):
                        cc = q * QC + c
                        nc.tensor.matmul(ps_c[:], ohlo[:, c], rhs[:, c],
                                         start=(cc == 0), stop=(cc == 127))
                        nc.tensor.matmul(ps_d[:], ohlod[:, c], rhsd[:, c],
                                         start=(cc == 0), stop=(cc == 127))

                # tables: h[lo, 0:32]=total, h[lo, 32:64]=fore counts
                h = timg.tile([128, 64], F32)
                v.tensor_copy(h[:], ps_c[:])
                hd = timg.tile([16, 32], F32)
                v.tensor_copy(hd[:], ps_d[:])
                colsum = timg.tile([128, 1], F32)
                v.tensor_reduce(colsum[:], h[:, 32:64], mybir.AxisListType.X, AL.add)
                nf_ps = psb.tile([1, 1], F32, tag="ps_small")
                nc.tensor.matmul(nf_ps[:], on128[:], colsum[:],
                                 start=True, stop=True)
                nf_sb = timg.tile([1, 1], F32)
                v.tensor_copy(nf_sb[:], nf_ps[:])
                ab_st = timg.tile([1, 2], F32)
                v.tensor_scalar(ab_st[:, 0:1], nf_sb[:], 1.0, None, op0=AL.add)
                v.tensor_scalar(ab_st[:, 1:2], nf_sb[:], -1.0, float(NT + 1),
                                op0=AL.mult, op1=AL.add)
                v.reciprocal(ab_st[:], ab_st[:])
                ab_ps = psb.tile([128, 2], F32, tag="ps_small")
                nc.tensor.matmul(ab_ps[:], on1[:], ab_st[:], start=True, stop=True)

                def make_table(hh, nlo, nhi, dram_dst):
                    cb = timg.tile([nlo, nhi], F32, tag="cb")
                    v.tensor_tensor(cb[:], hh[:, 0:nhi], hh[:, nhi:2 * nhi],
                                    AL.subtract)
                    fn = timg.tile([nlo, nhi], F32, tag="fn")
                    v.tensor_scalar(fn[:], hh[:, nhi:2 * nhi],
                                    ab_ps[0:nlo, 0:1], None, op0=AL.mult)
                    bn = timg.tile([nlo, nhi], F32, tag="bn")
                    v.tensor_scalar(bn[:], cb[:], ab_ps[0:nlo, 1:2], None,
                                    op0=AL.mult)
                    den = timg.tile([nlo, nhi], F32, tag="den")
                    v.tensor_tensor(den[:], fn[:], bn[:], AL.add)
                    v.tensor_scalar(den[:], den[:], EPS, None, op0=AL.add)
                    v.reciprocal(den[:], den[:])
                    stage = timg.tile([nlo, nhi, 2], F32, tag="stage")
                    v.tensor_tensor(stage[:, :, 0], fn[:], den[:], AL.mult)
                    v.tensor_tensor(stage[:, :, 1], bn[:], den[:], AL.mult)
                    nc.sync.dma_start(dram_dst, stage[:].rearrange("p a b -> p (a b)"))

                make_table(h, 128, 32, tblc_d[b].rearrange("(p f) -> p f", p=128))
                make_table(hd, 16, 16, tbld_d[b].rearrange("(p f) -> p f", p=16))

                gt_c = gtbl.tile([128, 8192], F32, tag="gt_c")
                nc.sync.dma_start(gt_c[0:128:16, :],
                                  tblc_d[b].unsqueeze(0).to_broadcast([8, 8192]))
                gt_d = gtbl.tile([128, 512], F32, tag="gt_d")
                nc.sync.dma_start(gt_d[0:128:16, :],
                                  tbld_d[b].unsqueeze(0).to_broadcast([8, 512]))

                # ---------------- search phase ----------------
                # partition 16c+p holds pixels c*18432 + s*16 + p (half-image
                # at a time) -> gather output cols are raster-contiguous/core
                HPX = NPX // 2          # pixels per half
                HS = PPP // 2           # slots per half (576)
                for half in range(2):
                    sc_t = spx.tile([128, 3 * HS], I32, tag="sc")
                    sd_t = spx.tile([128, HS], I32, tag="sd")
                    for c in range(8):
                        base = half * (HPX // 8) + c * (NPX // 8)
                        src_c = scol[b, 3 * base:3 * (base + HPX // 8)]
                        src_c = src_c.rearrange("(srs p ch) -> srs p ch",
                                                p=16, ch=3).transpose([1, 0, 2])
                        dst_c = sc_t[16 * c:16 * (c + 1), :].rearrange(
                            "p (srs ch) -> p srs ch", ch=3)
                        nc.sync.dma_start(dst_c, src_c)
                        src_d = sdep[b, base:base + HPX // 8]
                        src_d = src_d.rearrange("(srs p) -> srs p",
                                                p=16).transpose([1, 0])
                        nc.sync.dma_start(sd_t[16 * c:16 * (c + 1), :], src_d)
                    sch = sc_t[:].rearrange("p (w c) -> p w c", c=3)
                    for kk in range(HS // S_COLS):
                        ss = slice(S_COLS * kk, S_COLS * (kk + 1))
                        sch_k = sch[:, ss]
                        # color idx' = (g&112)<<5 | (b&240)<<1 | (r&240)>>3 | g>>7
                        x0 = spx.tile([128, S_COLS], I32, tag="x0")
                        v.tensor_scalar(x0[:], sch_k[:, :, 0], 240, 3,
                                        op0=AL.bitwise_and,
                                        op1=AL.logical_shift_right)
                        x1 = spx.tile([128, S_COLS], I32, tag="x1")
                        v.tensor_scalar(x1[:], sch_k[:, :, 1], 112, 5,
                                        op0=AL.bitwise_and,
                                        op1=AL.logical_shift_left)
                        x2 = spx.tile([128, S_COLS], I32, tag="x2")
                        v.tensor_scalar(x2[:], sch_k[:, :, 2], 240, 1,
                                        op0=AL.bitwise_and,
                                        op1=AL.logical_shift_left)
                        x3 = spx.tile([128, S_COLS], I32, tag="x3")
                        v.tensor_scalar(x3[:], sch_k[:, :, 1], 7, None,
                                        op0=AL.logical_shift_right)
                        v.tensor_tensor(x0[:], x0[:], x1[:], AL.add)
                        v.tensor_tensor(x2[:], x2[:], x3[:], AL.add)
                        idxc = spx.tile([128, S_COLS], I16, tag="idxc")
                        v.tensor_tensor(idxc[:], x0[:], x2[:], AL.add)
                        # depth idx' = (d&15)<<4 | d>>4
                        y0 = spx.tile([128, S_COLS], I32, tag="y0")
                        v.tensor_scalar(y0[:], sd_t[:, ss], 15, 4,
                                        op0=AL.bitwise_and,
                                        op1=AL.logical_shift_left)
                        y1 = spx.tile([128, S_COLS], I32, tag="y1")
                        v.tensor_scalar(y1[:], sd_t[:, ss], 4, None,
                                        op0=AL.logical_shift_right)
                        idxd = spx.tile([128, S_COLS], I16, tag="idxd")
                        v.tensor_tensor(idxd[:], y0[:], y1[:], AL.add)

                        gc = gpoolo.tile([128, NIDX, 2], F32, tag="gc")
                        nc.gpsimd.ap_gather(
                            gc[:], gt_c[:].rearrange("p (n d) -> p n d", d=2),
                            idxc[:], channels=128, num_elems=4096,
                            d=2, num_idxs=NIDX)
                        gd = gpoolo.tile([128, NIDX, 2], F32, tag="gd")
                        nc.gpsimd.ap_gather(
                            gd[:], gt_d[:].rearrange("p (n d) -> p n d", d=2),
                            idxd[:], channels=128, num_elems=256,
                            d=2, num_idxs=NIDX)

                        off = half * (HPX // 8) + kk * NIDX
                        for ch, (g, fb) in enumerate([(gc, 0), (gc, 1),
                                                      (gd, 0), (gd, 1)]):
                            src = g[0:128:16, :, fb]
                            dst = out[b, ch].rearrange("(c j) -> c j", c=8)
                            dst = dst[:, off:off + NIDX]
                            nc.sync.dma_start(dst, src)
    nc.compile()
    return nc


_NC_CACHE = None


def _get_nc():
    global _NC_CACHE
    if _NC_CACHE is None:
        _NC_CACHE = _build_nc()
    return _NC_CACHE


def _consts():
    i = np.arange(128, dtype=np.float32)
    import ml_dtypes
    return {
        "iota128": np.broadcast_to(i[None, :128], (128, 128)).astype(ml_dtypes.bfloat16),
        "iota32": np.broadcast_to(i[None, :32], (128, 32)).astype(ml_dtypes.bfloat16),
        "iota16": np.broadcast_to(i[None, :16], (128, 16)).astype(ml_dtypes.bfloat16),
        "iotap": i[:, None].copy(),
        "iotac": np.broadcast_to(i[None, :128], (128, 128)).astype(np.float32).copy(),
        "ones1": np.ones((1, 128), np.float32),
        "ones128": np.ones((128, 1), np.float32),
    }


def kernel(target_anno, template_color, search_color, template_depth, search_depth):
    nc = _get_nc()
    consts = _consts()
    in_maps = []
    for c in range(NCORES):
        s = slice(c * BPC, (c + 1) * BPC)
        in_maps.append({
            "anno": np.ascontiguousarray(target_anno[s]).astype(np.float32),
            "tcol": np.ascontiguousarray(
                template_color[s].reshape(BPC, 128, 3 * Wt)).astype(np.int32),
            "tdep": np.ascontiguousarray(
                template_depth[s].reshape(BPC, 128, Wt)).astype(np.int32),
            "scol": np.ascontiguousarray(
                search_color[s].reshape(BPC, 3 * NPX)).astype(np.int32),
            "sdep": np.ascontiguousarray(
                search_depth[s].reshape(BPC, NPX)).astype(np.int32),
            **consts,
        })
    res = bass_utils.run_bass_kernel_spmd(nc, in_maps, core_ids=list(range(NCORES)))
    outs = [res.results[c]["out"].reshape(BPC, 4, Hs, Ws) for c in range(NCORES)]
    return np.concatenate(outs, axis=0)
